# revision 56
# baseline (speedup 1.0000x reference)
"""Trainium2 Bass kernel for MultiHeadLatentAttention.

Problem (hardcoded): B=2, S=2048, DIN=2048, DOUT=2048, LATENT=512,
HEADS=16, head_dim=128, fp32 in/out, causal attention, softmax scale
1/sqrt(S).

Sharding: 8 cores = batch (2) x head-groups (4 groups of 4 heads).
Each core computes, for its (batch b, head group g):
    q = x_b @ Wq[:, g]            (as q^T, head-dim on partitions, RoPE'd)
    kv_lat = x_b @ Wl             (as kv_lat^T)
    k^T = Wu_k[:, g]^T @ kv_lat^T (RoPE'd), v = kv_lat @ Wu_v[:, g]
    per head: E^T = exp(scale * K Q^T) (causal), O^T = V^T E^T / R
    pout = O @ Wp[g rows, :]      (partial over head-group dims)
Host sums the 4 per-group partials for each batch.

All on-device data is fp16 (inputs are ~N(0,1)-scaled so fp16 quant
error ~1e-3 << the 2e-2 gate); matmuls run at full PE rate with FWL
weight loads. Single pass over x: each 512-wide s-block computes
kv_latent, K^T (roped), V, and Q (roped); attention chunk qi is
emitted as soon as s-blocks 0..qi are done so its exp/mask work rides
under later projection matmuls. Output projection is fused into the
attention tail.
"""

import math
import os

import numpy as np

import concourse.bass as bass
import concourse.mybir as mybir
import concourse.tile as tile
from concourse import bacc, bass_utils

# ---- problem constants (self-contained; do not read spec/reference) ----
B = 2
S = 2048
DIN = 2048
DOUT = 2048
LATENT = 512
HEADS = 16
HD = 128                 # head dim
NCORES = 8
GROUPS = 4               # head groups (tensor parallel dimension)
GH = HEADS // GROUPS     # heads per group = 4
GD = GH * HD             # dims per group = 512

SB = 512                 # s-block width (also attention q-chunk width)
NSB = S // SB            # 4
KT = DIN // 128          # 16 contraction tiles over DIN
LT = LATENT // 128       # 4 contraction tiles over LATENT
QC = 512                 # q-chunk width in attention
NQC = S // QC            # 4

F32 = mybir.dt.float32
F16 = mybir.dt.float16
F8 = mybir.dt.float8e4
SCALE = 1.0 / math.sqrt(float(S))


def build_nc(stage=None, repeat=None):
    if stage is None:
        stage = int(os.environ.get("K_STAGE", "4"))
    if repeat is None:
        repeat = int(os.environ.get("K_REPEAT", "1"))
    nc = bacc.Bacc(
        "TRN2", target_bir_lowering=False, debug=False, num_devices=NCORES
    )
    _build_body(nc, stage, repeat)
    nc.compile()
    return nc


def _build_body(nc, stage, repeat=1):
    xT = nc.dram_tensor("xT", [DIN, S], F16, kind="ExternalInput")
    xq8 = nc.dram_tensor("xq8", [DIN, S], F8, kind="ExternalInput")
    wq = nc.dram_tensor("wq", [DIN, GD], F8, kind="ExternalInput")
    wl = nc.dram_tensor("wl", [DIN, LATENT], F16, kind="ExternalInput")
    wuk = nc.dram_tensor("wuk", [LATENT, GD], F16, kind="ExternalInput")
    wuv = nc.dram_tensor("wuv", [LATENT, GD], F16, kind="ExternalInput")
    wp = nc.dram_tensor("wp", [GD, DOUT], F16, kind="ExternalInput")
    cosT = nc.dram_tensor("cosT", [HD, S], F32, kind="ExternalInput")
    sinT = nc.dram_tensor("sinT", [HD, S], F32, kind="ExternalInput")
    masksd = nc.dram_tensor("masks", [128, QC + 384], F16, kind="ExternalInput")
    eyed = nc.dram_tensor("eye", [128, 128], F16, kind="ExternalInput")
    pout = nc.dram_tensor("pout", [S, DOUT], F16, kind="ExternalOutput")

    xT_t = xT.rearrange("(ko ki) s -> ki ko s", ki=128)       # [128,16,S]
    xq8_t = xq8.rearrange("(ko ki) s -> ki ko s", ki=128)     # [128,16,S]
    wq_t = wq.rearrange("(ko ki) d -> ki ko d", ki=128)       # [128,16,GD]
    wl_t = wl.rearrange("(ko ki) l -> ki ko l", ki=128)       # [128,16,LAT]
    wuk_t = wuk.rearrange("(lo li) d -> li lo d", li=128)     # [128,4,GD]
    wuv_t = wuv.rearrange("(lo li) d -> li lo d", li=128)     # [128,4,GD]
    wp_t = wp.rearrange("(dt di) e -> di dt e", di=128)       # [128,4,DOUT]

    with tile.TileContext(nc) as tc:
      for _rep in range(repeat):
        with (
            tc.tile_pool(name="persist", bufs=1) as persist,
            tc.tile_pool(name="kvres", bufs=1) as kvres,
        ):
            # manually-released pools (right-side stack, LIFO)
            xtp = tc.alloc_tile_pool(name="xt", bufs=2, side="right")
            cs = tc.alloc_tile_pool(name="cs", bufs=1, side="right")
            cos_sb = cs.tile([HD, S], F32)
            sin_sb = cs.tile([HD, S], F32)
            eye_sb = persist.tile([128, 128], F16)
            masks_sb = persist.tile([128, QC + 384], F16)

            # persistent per-chunk state (chunk = 512 seq positions)
            kT_c = [kvres.tile([128, GH, QC], F16, tag=f"kT{c}", name=f"kT{c}")
                    for c in range(NQC)]
            qT_c = [kvres.tile([128, GH, QC], F16, tag=f"qT{c}", name=f"qT{c}")
                    for c in range(NQC)]
            # V in fp16 with a ones column appended per head: feeds the
            # ones-column AV matmul that yields O and the softmax sum R
            v_c = [kvres.tile([128, 4, GH, HD + 1], F16,
                              tag=f"v{c}", name=f"v{c}") for c in range(NQC)]
            # normalized attention output per q-chunk, [q, s-sub, d]
            o_c = [kvres.tile([128, 4, GD], F16, tag=f"o{c}", name=f"o{c}")
                   for c in range(NQC)]
            for c in range(NQC):
                nc.vector.memset(v_c[c][:, :, :, HD:], 1.0)

            def rope(dst, src_ps, tmp_pool, s0, n):
                """dst[:] = rope(src_ps) using cos/sin slices [s0:s0+n]."""
                tmp = tmp_pool.tile([128, SB], F32, tag="rope_tmp")
                nc.vector.tensor_mul(
                    tmp[0:64, :n], src_ps[64:128, :], sin_sb[0:64, s0:s0 + n]
                )
                nc.vector.tensor_mul(
                    tmp[64:128, :n], src_ps[0:64, :], sin_sb[64:128, s0:s0 + n]
                )
                nc.vector.tensor_mul(dst, src_ps[:, :], cos_sb[:, s0:s0 + n])
                nc.gpsimd.tensor_add(dst, dst, tmp[:, :n])

            with (
                tc.tile_pool(name="w1", bufs=1) as w1,
                tc.tile_pool(name="kvl", bufs=3) as kvlp,
                tc.tile_pool(name="tmp1", bufs=3) as tmp1,
                tc.tile_pool(name="att", bufs=4) as attp,
                tc.tile_pool(name="ps_lg", bufs=2, space="PSUM") as pslg,
                tc.tile_pool(name="ps_ot", bufs=1, space="PSUM") as psot,
            ):
                # proj psums released before stage 2 (PSUM bank budget)
                ps1 = tc.alloc_tile_pool(name="ps1", bufs=4, space="PSUM",
                                         side="right")
                wl_sb = w1.tile([128, KT, LATENT], F16)
                wq_sb = w1.tile([128, KT, GD], F8)
                wuk_sb = w1.tile([128, LT, GD], F16)
                wuv_sb = w1.tile([128, LT, GD], F16)

                # ---- DMA issue order = arrival order (single queue) ----
                # first x chunk + first wl chunks gate the first matmul
                xt_sbs = [None] * NSB
                xq_sbs = [None] * NSB
                xt_sbs[0] = xtp.tile([128, KT, SB], F16, tag="xt", name="xt0")
                xq_sbs[0] = xtp.tile([128, KT, SB], F8, tag="xq", name="xq0")
                # growing-size first-block loads: the first matmuls' data
                # arrives after 4 small dispatches; the rest in 4 larger
                # ones (each Sync dispatch costs ~0.6us, so fine-grained
                # splitting of everything would delay the later chunks)
                for ko in range(2):
                    nc.sync.dma_start(xt_sbs[0][:, ko, :],
                                      xT_t[:, ko, 0:SB])
                    nc.sync.dma_start(wl_sb[:, ko, :], wl_t[:, ko, :])
                nc.sync.dma_start(xt_sbs[0][:, 2:8, :], xT_t[:, 2:8, 0:SB])
                nc.sync.dma_start(wl_sb[:, 2:8, :], wl_t[:, 2:8, :])
                nc.sync.dma_start(xt_sbs[0][:, 8:, :], xT_t[:, 8:, 0:SB])
                nc.sync.dma_start(wl_sb[:, 8:, :], wl_t[:, 8:, :])
                nc.sync.dma_start(wuk_sb[:], wuk_t)
                nc.sync.dma_start(wuv_sb[:], wuv_t)
                nc.sync.dma_start(cos_sb[:], cosT[:, :])
                nc.sync.dma_start(sin_sb[:], sinT[:, :])
                nc.sync.dma_start(wq_sb[:], wq_t)
                nc.sync.dma_start(xq_sbs[0][:], xq8_t[:, :, 0:SB])
                nc.sync.dma_start(masks_sb[:], masksd[:, :])
                nc.sync.dma_start(eye_sb[:], eyed[:, :])

                def proj_block(sb, fillers=()):
                    """kv_latent, K^T(+rope), V, Q(+rope) for s-block sb.

                    fillers: emitted between sections; attention heads of
                    the previous chunk go here so their exp latency hides
                    under projection matmuls (and projection PSUM-drain
                    stalls hide under attention matmuls)."""
                    fillers = list(fillers)

                    def fill():
                        if fillers:
                            fillers.pop(0)()

                    s0 = sb * SB
                    xt_sb = xt_sbs[sb]

                    def q_section():
                        # Q per head (+rope): fp8 DoubleRow, 2 k-planes/MM
                        xq_sb = xq_sbs[sb]
                        for hh in range(GH):
                            ps = ps1.tile([128, SB], F32, tag="ps1")
                            for ko in range(0, KT, 2):
                                nc.tensor.matmul(
                                    ps[:],
                                    wq_sb[:, ko:ko + 2,
                                          hh * 128:(hh + 1) * 128],
                                    xq_sb[:, ko:ko + 2, :],
                                    start=(ko == 0),
                                    stop=(ko == KT - 2),
                                    perf_mode=mybir.MatmulPerfMode.DoubleRow,
                                )
                            rope(qT_c[sb][:, hh, :], ps, tmp1, s0, SB)
                        fill()

                    # kv_latent^T [128, LT, SB]
                    kvl_sb = kvlp.tile([128, LT, SB], F16, tag="kvl")
                    for lo in range(LT):
                        ps = ps1.tile([128, SB], F32, tag="ps1")
                        for ko in range(KT):
                            nc.tensor.matmul(
                                ps[:],
                                wl_sb[:, ko, lo * 128:(lo + 1) * 128],
                                xt_sb[:, ko, :],
                                start=(ko == 0),
                                stop=(ko == KT - 1),
                            )
                        nc.vector.tensor_copy(kvl_sb[:, lo, :], ps[:])
                    fill()

                    # K^T per head (+rope)
                    for hh in range(GH):
                        ps = ps1.tile([128, SB], F32, tag="ps1")
                        for lo in range(LT):
                            nc.tensor.matmul(
                                ps[:],
                                wuk_sb[:, lo, hh * 128:(hh + 1) * 128],
                                kvl_sb[:, lo, :],
                                start=(lo == 0),
                                stop=(lo == LT - 1),
                            )
                        rope(kT_c[sb][:, hh, :], ps, tmp1, s0, SB)
                    fill()

                    # V in [s, d] layout: s-chunks of 128
                    for sc in range(4):
                        ps = ps1.tile([128, GD], F32, tag="ps1")
                        for lo in range(LT):
                            nc.tensor.matmul(
                                ps[:],
                                kvl_sb[:, lo, sc * 128:(sc + 1) * 128],
                                wuv_sb[:, lo, :],
                                start=(lo == 0),
                                stop=(lo == LT - 1),
                            )
                        nc.scalar.copy(
                            v_c[sb][:, sc, :, :HD],
                            ps.rearrange("p (h d) -> p h d", h=GH),
                        )
                    fill()

                    q_section()

                def attn(qi, heads=range(GH)):
                    njb = 4 * qi + 4  # causal: k-blocks 0..4*qi+3

                    def qk_exp(j, hh):
                        """logits + exp(+causal mask) for k-block j."""
                        t = j - 4 * qi
                        # causal: q-cols < 128*t fully masked
                        qoff = 0 if t < 1 else 128 * t
                        nw = QC - qoff
                        lg = pslg.tile([128, QC], F32, tag="lg")
                        nc.tensor.matmul(
                            lg[:, :nw],
                            kT_c[j // 4][:, hh,
                                         (j % 4) * 128:(j % 4 + 1) * 128],
                            qT_c[qi][:, hh, qoff:],
                            start=True,
                            stop=True,
                        )
                        e_sb = attp.tile([128, QC], F16, tag="e")
                        nc.scalar.activation(
                            e_sb[:, :nw],
                            lg[:, :nw],
                            mybir.ActivationFunctionType.Exp,
                            scale=SCALE,
                        )
                        if t >= 0:
                            m0 = 384 - 128 * t + qoff
                            nc.vector.tensor_mul(
                                e_sb[:, :nw], e_sb[:, :nw],
                                masks_sb[:, m0:m0 + nw],
                            )
                        return e_sb

                    for hh in heads:
                        # [128,129] accumulator per 128-wide q-sub, packed
                        # in pairs so each stays inside one PSUM bank:
                        # cols 0:128 = O (q rows, d cols), col 128 = R
                        oqA = psot.tile([128, 2 * (HD + 1)], F32, tag="oqA",
                                        bufs=1, name="oqA")
                        oqB = psot.tile([128, 2 * (HD + 1)], F32, tag="oqB",
                                        bufs=1, name="oqB")
                        o_qs = [oqA[:, 0:HD + 1], oqA[:, HD + 1:],
                                oqB[:, 0:HD + 1], oqB[:, HD + 1:]]
                        # software pipeline: QK/exp two k-blocks ahead of AV
                        e_q = [qk_exp(jj, hh) for jj in range(min(2, njb))]
                        for j in range(njb):
                            t = j - 4 * qi
                            qoff = 0 if t < 1 else 128 * t
                            e_sb = e_q.pop(0)
                            if j + 2 < njb:
                                e_q.append(qk_exp(j + 2, hh))
                            for s in range(max(t, 0), 4):
                                # start=True clears the whole PSUM bank, so
                                # only the bank's first matmul (s even at
                                # j=0) gets it; the odd-s group's first
                                # write lands on has_written=0 elements and
                                # overwrites rather than accumulates.
                                nc.tensor.matmul(
                                    o_qs[s][:, :],
                                    e_sb[:, s * 128 - qoff:
                                         (s + 1) * 128 - qoff],
                                    v_c[j // 4][:, j % 4, hh, :],
                                    start=(j == 0 and s % 2 == 0),
                                    stop=(j == 4 * qi + s),
                                )
                        # normalize rows: O[q, :] / R[q] (per-partition
                        # scalar); runs on ACT so the next head's AV can
                        # start into the other PSUM buffer immediately
                        for s in range(4):
                            rec = attp.tile([128, 1], F32, tag="rec")
                            nc.vector.reciprocal(rec[:], o_qs[s][:, HD:])
                            nc.scalar.mul(
                                o_c[qi][:, s, hh * HD:(hh + 1) * HD],
                                o_qs[s][:, :HD],
                                rec[:],
                            )

                # ---- stage 1: s-blocks with attention chunks woven in ----
                for sb in range(NSB):
                    if sb + 1 < NSB:
                        # xq8 first: the next block opens with its q section
                        nxq = xtp.tile([128, KT, SB], F8, tag="xq",
                                       name=f"xq{sb + 1}")
                        xq_sbs[sb + 1] = nxq
                        nc.sync.dma_start(
                            nxq[:], xq8_t[:, :, (sb + 1) * SB:(sb + 2) * SB])
                        nxt = xtp.tile([128, KT, SB], F16, tag="xt",
                                       name=f"xt{sb + 1}")
                        xt_sbs[sb + 1] = nxt
                        for kg in range(4):
                            nc.sync.dma_start(
                                nxt[:, 4 * kg:4 * kg + 4, :],
                                xT_t[:, 4 * kg:4 * kg + 4,
                                     (sb + 1) * SB:(sb + 2) * SB],
                            )
                    if sb == 0:
                        proj_block(sb)
                    else:
                        proj_block(sb, [
                            (lambda h: lambda: attn(sb - 1, heads=[h]))(h)
                            for h in range(GH)
                        ])

                if stage <= 1:
                    nc.sync.dma_start(pout[0:128, 0:516], v_c[0][:, 0, :, :])
                    nc.sync.dma_start(pout[128:256, 0:512],
                                      kT_c[0][:, 0, 0:512])
                    ps1.release()
                    cs.release()
                    xtp.release()
                    return

                # ---- stage 2: attn(3) + all output projections fused ----
                ps1.release()
                cs.release()
                xtp.release()
                with (
                    tc.tile_pool(name="w3", bufs=1) as w3,
                    tc.tile_pool(name="otr", bufs=3) as otrp,
                    tc.tile_pool(name="osb", bufs=4) as osbp,
                    tc.tile_pool(name="ps3", bufs=1, space="PSUM",
                                 side="right") as ps3,
                ):
                    wp_sb = w3.tile([128, LT, DOUT], F16)
                    for dt_ in range(LT):
                        nc.sync.dma_start(wp_sb[:, dt_, :], wp_t[:, dt_, :])

                    def ph3(qi, sc):
                        q0 = qi * QC
                        tr = ps3.tile([128, 512], F16, tag="tr", bufs=2)
                        for dt_ in range(LT):
                            nc.tensor.transpose(
                                tr[:, dt_ * HD:(dt_ + 1) * HD],
                                o_c[qi][:, sc, dt_ * HD:(dt_ + 1) * HD],
                                eye_sb[:],
                            )
                        # after attention drains, ACT is idle: split the
                        # tail chunk's PSUM evacuations across ACT and DVE
                        def evac(o, i, k):
                            if qi == 3 and k % 2 == 0:
                                nc.scalar.copy(o, i)
                            else:
                                nc.vector.tensor_copy(o, i)
                        oT_sb = otrp.tile([128, 512], F16, tag="ot")
                        evac(oT_sb[:], tr[:], 0)
                        out_sb = osbp.tile([128, DOUT], F16, tag="out")
                        for ec in range(DOUT // 512):
                            po = ps3.tile([128, 512], F32, tag="po", bufs=2)
                            for dt_ in range(LT):
                                nc.tensor.matmul(
                                    po[:],
                                    oT_sb[:, dt_ * HD:(dt_ + 1) * HD],
                                    wp_sb[:, dt_, ec * 512:(ec + 1) * 512],
                                    start=(dt_ == 0),
                                    stop=(dt_ == LT - 1),
                                )
                            evac(out_sb[:, ec * 512:(ec + 1) * 512], po[:],
                                 ec + 1)
                        nc.sync.dma_start(
                            pout[q0 + sc * 128:q0 + (sc + 1) * 128, :],
                            out_sb[:],
                        )

                    # attn(3) heads interleaved with the already-finished
                    # chunks' output projections (fills ACT-bound PE gaps)
                    units = [(qi, sc) for qi in range(3) for sc in range(4)]
                    for x in range(4):
                        attn(3, heads=[x])
                        for qi, sc in units[3 * x:3 * x + 3]:
                            ph3(qi, sc)
                    for sc in range(4):
                        ph3(3, sc)
            # (kvres/persist close)


_CACHE: dict = {}


def _get_nc():
    if "nc" not in _CACHE:
        _CACHE["nc"] = build_nc()
    return _CACHE["nc"]


def _host_inputs(x, position_embeddings, Wq, Wl, Wu, Wp):
    x = np.asarray(x, dtype=np.float32)
    pe = np.asarray(position_embeddings, dtype=np.float32)[:S]
    Wq = np.asarray(Wq, dtype=np.float32)
    Wl = np.asarray(Wl, dtype=np.float32)
    Wu = np.asarray(Wu, dtype=np.float32)
    Wp = np.asarray(Wp, dtype=np.float32)

    cos = np.ascontiguousarray(np.cos(pe).T)          # [HD, S]
    sinF = np.ascontiguousarray(np.sin(pe).T)         # [HD, S]
    sinF[: HD // 2] *= -1.0                           # fold rotate-half sign

    k = np.arange(128)[:, None]
    c = np.arange(QC + 384)[None, :]
    masks = np.ascontiguousarray((c - 384 >= k).astype(np.float16))

    import ml_dtypes

    f8 = ml_dtypes.float8_e4m3
    xTs = [np.ascontiguousarray(x[b].T.astype(np.float16)) for b in range(B)]
    xq8s = [np.ascontiguousarray(x[b].T.astype(f8)) for b in range(B)]
    wl16 = np.ascontiguousarray(Wl.astype(np.float16))

    in_maps = []
    for cid in range(NCORES):
        b, g = divmod(cid, GROUPS)
        in_maps.append({
            "xT": xTs[b],
            "xq8": xq8s[b],
            "wq": np.ascontiguousarray(
                Wq[:, g * GD:(g + 1) * GD].astype(f8)),
            "wl": wl16,
            "wuk": np.ascontiguousarray(
                Wu[:, g * GD:(g + 1) * GD].astype(np.float16)),
            "wuv": np.ascontiguousarray(
                Wu[:, DOUT + g * GD:DOUT + (g + 1) * GD].astype(np.float16)),
            "wp": np.ascontiguousarray(
                Wp[g * GD:(g + 1) * GD, :].astype(np.float16)),
            "cosT": cos,
            "sinT": sinF,
            "masks": masks,
            "eye": np.eye(128, dtype=np.float16),
        })
    return in_maps


def run(x, position_embeddings, Wq, Wl, Wu, Wp, trace=False, trace_cores=None):
    """Run on 8 cores; returns (output, BassKernelResults)."""
    nc = _get_nc()
    in_maps = _host_inputs(x, position_embeddings, Wq, Wl, Wu, Wp)
    if trace and trace_cores is None:
        trace_cores = list(range(NCORES))
    res = bass_utils.run_bass_kernel_spmd(
        nc, in_maps, core_ids=list(range(NCORES)), trace=trace,
        trace_cores=trace_cores,
    )
    parts = [r["pout"].astype(np.float32) for r in res.results]
    out = np.empty((B, S, DOUT), dtype=np.float32)
    for b in range(B):
        out[b] = np.sum(
            np.stack(parts[b * GROUPS:(b + 1) * GROUPS]),
            axis=0, dtype=np.float64,
        ).astype(np.float32)
    return out, res


def kernel(x, position_embeddings, Wq, Wl, Wu, Wp):
    out, _ = run(x, position_embeddings, Wq, Wl, Wu, Wp, trace=False)
    return out


# revision 62
# speedup vs baseline: 1.0125x; 1.0125x over previous
"""Trainium2 Bass kernel for MultiHeadLatentAttention.

Problem (hardcoded): B=2, S=2048, DIN=2048, DOUT=2048, LATENT=512,
HEADS=16, head_dim=128, fp32 in/out, causal attention, softmax scale
1/sqrt(S).

Sharding: 8 cores = batch (2) x head-groups (4 groups of 4 heads).
Each core computes, for its (batch b, head group g):
    q = x_b @ Wq[:, g]            (as q^T, head-dim on partitions, RoPE'd)
    kv_lat = x_b @ Wl             (as kv_lat^T)
    k^T = Wu_k[:, g]^T @ kv_lat^T (RoPE'd), v = kv_lat @ Wu_v[:, g]
    per head: E^T = exp(scale * K Q^T) (causal), O^T = V^T E^T / R
    pout = O @ Wp[g rows, :]      (partial over head-group dims)
Host sums the 4 per-group partials for each batch.

All on-device data is fp16 (inputs are ~N(0,1)-scaled so fp16 quant
error ~1e-3 << the 2e-2 gate); matmuls run at full PE rate with FWL
weight loads. Single pass over x: each 512-wide s-block computes
kv_latent, K^T (roped), V, and Q (roped); attention chunk qi is
emitted as soon as s-blocks 0..qi are done so its exp/mask work rides
under later projection matmuls. Output projection is fused into the
attention tail.
"""

import math
import os

import numpy as np

import concourse.bass as bass
import concourse.mybir as mybir
import concourse.tile as tile
from concourse import bacc, bass_utils

# ---- problem constants (self-contained; do not read spec/reference) ----
B = 2
S = 2048
DIN = 2048
DOUT = 2048
LATENT = 512
HEADS = 16
HD = 128                 # head dim
NCORES = 8
GROUPS = 4               # head groups (tensor parallel dimension)
GH = HEADS // GROUPS     # heads per group = 4
GD = GH * HD             # dims per group = 512

SB = 512                 # s-block width (also attention q-chunk width)
NSB = S // SB            # 4
KT = DIN // 128          # 16 contraction tiles over DIN
LT = LATENT // 128       # 4 contraction tiles over LATENT
QC = 512                 # q-chunk width in attention
NQC = S // QC            # 4

F32 = mybir.dt.float32
F16 = mybir.dt.float16
F8 = mybir.dt.float8e4
SCALE = 1.0 / math.sqrt(float(S))


def build_nc(stage=None, repeat=None):
    if stage is None:
        stage = int(os.environ.get("K_STAGE", "4"))
    if repeat is None:
        repeat = int(os.environ.get("K_REPEAT", "1"))
    nc = bacc.Bacc(
        "TRN2", target_bir_lowering=False, debug=False, num_devices=NCORES
    )
    _build_body(nc, stage, repeat)
    nc.compile()
    return nc


def _build_body(nc, stage, repeat=1):
    xT = nc.dram_tensor("xT", [DIN, S], F16, kind="ExternalInput")
    xq8 = nc.dram_tensor("xq8", [DIN, S], F8, kind="ExternalInput")
    wq = nc.dram_tensor("wq", [DIN, GD], F8, kind="ExternalInput")
    wl = nc.dram_tensor("wl", [DIN, LATENT], F16, kind="ExternalInput")
    wuk = nc.dram_tensor("wuk", [LATENT, GD], F16, kind="ExternalInput")
    wuv = nc.dram_tensor("wuv", [LATENT, GD], F16, kind="ExternalInput")
    wp = nc.dram_tensor("wp", [GD, DOUT], F16, kind="ExternalInput")
    cosT = nc.dram_tensor("cosT", [HD, S], F32, kind="ExternalInput")
    sinT = nc.dram_tensor("sinT", [HD, S], F32, kind="ExternalInput")
    masksd = nc.dram_tensor("masks", [128, QC + 384], F16, kind="ExternalInput")
    eyed = nc.dram_tensor("eye", [128, 128], F16, kind="ExternalInput")
    pout = nc.dram_tensor("pout", [S, DOUT], F16, kind="ExternalOutput")

    xT_t = xT.rearrange("(ko ki) s -> ki ko s", ki=128)       # [128,16,S]
    xq8_t = xq8.rearrange("(ko ki) s -> ki ko s", ki=128)     # [128,16,S]
    wq_t = wq.rearrange("(ko ki) d -> ki ko d", ki=128)       # [128,16,GD]
    wl_t = wl.rearrange("(ko ki) l -> ki ko l", ki=128)       # [128,16,LAT]
    wuk_t = wuk.rearrange("(lo li) d -> li lo d", li=128)     # [128,4,GD]
    wuv_t = wuv.rearrange("(lo li) d -> li lo d", li=128)     # [128,4,GD]
    wp_t = wp.rearrange("(dt di) e -> di dt e", di=128)       # [128,4,DOUT]

    with tile.TileContext(nc) as tc:
      for _rep in range(repeat):
        with (
            tc.tile_pool(name="persist", bufs=1) as persist,
            tc.tile_pool(name="kvres", bufs=1) as kvres,
        ):
            # manually-released pools (right-side stack, LIFO)
            xtp = tc.alloc_tile_pool(name="xt", bufs=2, side="right")
            cs = tc.alloc_tile_pool(name="cs", bufs=1, side="right")
            cos_sb = cs.tile([HD, S], F32)
            sin_sb = cs.tile([HD, S], F32)
            eye_sb = persist.tile([128, 128], F16)
            masks_sb = persist.tile([128, QC + 384], F16)

            # persistent per-chunk state (chunk = 512 seq positions)
            kT_c = [kvres.tile([128, GH, QC], F16, tag=f"kT{c}", name=f"kT{c}")
                    for c in range(NQC)]
            qT_c = [kvres.tile([128, GH, QC], F16, tag=f"qT{c}", name=f"qT{c}")
                    for c in range(NQC)]
            # V in fp16 with a ones column appended per head: feeds the
            # ones-column AV matmul that yields O and the softmax sum R
            v_c = [kvres.tile([128, 4, GH, HD + 1], F16,
                              tag=f"v{c}", name=f"v{c}") for c in range(NQC)]
            # normalized attention output per q-chunk, [q, s-sub, d]
            o_c = [kvres.tile([128, 4, GD], F16, tag=f"o{c}", name=f"o{c}")
                   for c in range(NQC)]
            for c in range(NQC):
                nc.vector.memset(v_c[c][:, :, :, HD:], 1.0)

            def rope(dst, src_ps, tmp_pool, s0, n):
                """dst[:] = rope(src_ps) using cos/sin slices [s0:s0+n]."""
                tmp = tmp_pool.tile([128, SB], F32, tag="rope_tmp")
                nc.vector.tensor_mul(
                    tmp[0:64, :n], src_ps[64:128, :], sin_sb[0:64, s0:s0 + n]
                )
                nc.vector.tensor_mul(
                    tmp[64:128, :n], src_ps[0:64, :], sin_sb[64:128, s0:s0 + n]
                )
                nc.vector.tensor_mul(dst, src_ps[:, :], cos_sb[:, s0:s0 + n])
                nc.gpsimd.tensor_add(dst, dst, tmp[:, :n])

            with (
                tc.tile_pool(name="w1", bufs=1) as w1,
                tc.tile_pool(name="kvl", bufs=2) as kvlp,
                tc.tile_pool(name="tmp1", bufs=2) as tmp1,
                tc.tile_pool(name="att", bufs=3) as attp,
                tc.tile_pool(name="ps_lg", bufs=2, space="PSUM") as pslg,
                tc.tile_pool(name="ps_ot", bufs=1, space="PSUM") as psot,
            ):
                # proj psums released before stage 2 (PSUM bank budget)
                ps1 = tc.alloc_tile_pool(name="ps1", bufs=4, space="PSUM",
                                         side="right")
                wl_sb = w1.tile([128, KT, LATENT], F16)
                wq_sb = w1.tile([128, KT, GD], F8)
                wuk_sb = w1.tile([128, LT, GD], F16)
                wuv_sb = w1.tile([128, LT, GD], F16)

                # ---- DMA issue order = arrival order (single queue) ----
                # first x chunk + first wl chunks gate the first matmul
                xt_sbs = [None] * NSB
                xq_sbs = [None] * NSB
                xt_sbs[0] = xtp.tile([128, KT, SB], F16, tag="xt", name="xt0")
                xq_sbs[0] = xtp.tile([128, KT, SB], F8, tag="xq", name="xq0")
                # growing-size first-block loads: the first matmuls' data
                # arrives after 4 small dispatches; the rest in 4 larger
                # ones (each Sync dispatch costs ~0.6us, so fine-grained
                # splitting of everything would delay the later chunks)
                for ko in range(2):
                    nc.sync.dma_start(xt_sbs[0][:, ko, :],
                                      xT_t[:, ko, 0:SB])
                    nc.sync.dma_start(wl_sb[:, ko, :], wl_t[:, ko, :])
                nc.sync.dma_start(xt_sbs[0][:, 2:8, :], xT_t[:, 2:8, 0:SB])
                nc.sync.dma_start(wl_sb[:, 2:8, :], wl_t[:, 2:8, :])
                nc.sync.dma_start(xt_sbs[0][:, 8:, :], xT_t[:, 8:, 0:SB])
                nc.sync.dma_start(wl_sb[:, 8:, :], wl_t[:, 8:, :])
                nc.sync.dma_start(wuk_sb[:], wuk_t)
                nc.sync.dma_start(wuv_sb[:], wuv_t)
                nc.sync.dma_start(cos_sb[:], cosT[:, :])
                nc.sync.dma_start(sin_sb[:], sinT[:, :])
                nc.sync.dma_start(wq_sb[:], wq_t)
                nc.sync.dma_start(xq_sbs[0][:], xq8_t[:, :, 0:SB])
                nc.sync.dma_start(masks_sb[:], masksd[:, :])
                nc.sync.dma_start(eye_sb[:], eyed[:, :])

                def proj_block(sb, fillers=()):
                    """kv_latent, K^T(+rope), V, Q(+rope) for s-block sb.

                    fillers: emitted between sections; attention heads of
                    the previous chunk go here so their exp latency hides
                    under projection matmuls (and projection PSUM-drain
                    stalls hide under attention matmuls)."""
                    fillers = list(fillers)

                    def fill():
                        if fillers:
                            fillers.pop(0)()

                    s0 = sb * SB
                    xt_sb = xt_sbs[sb]

                    def q_section():
                        # Q per head (+rope): fp8 DoubleRow, 2 k-planes/MM
                        xq_sb = xq_sbs[sb]
                        for hh in range(GH):
                            ps = ps1.tile([128, SB], F32, tag="ps1")
                            for ko in range(0, KT, 2):
                                nc.tensor.matmul(
                                    ps[:],
                                    wq_sb[:, ko:ko + 2,
                                          hh * 128:(hh + 1) * 128],
                                    xq_sb[:, ko:ko + 2, :],
                                    start=(ko == 0),
                                    stop=(ko == KT - 2),
                                    perf_mode=mybir.MatmulPerfMode.DoubleRow,
                                )
                            rope(qT_c[sb][:, hh, :], ps, tmp1, s0, SB)
                        fill()

                    # kv_latent^T [128, LT, SB]
                    kvl_sb = kvlp.tile([128, LT, SB], F16, tag="kvl")
                    for lo in range(LT):
                        ps = ps1.tile([128, SB], F32, tag="ps1")
                        for ko in range(KT):
                            nc.tensor.matmul(
                                ps[:],
                                wl_sb[:, ko, lo * 128:(lo + 1) * 128],
                                xt_sb[:, ko, :],
                                start=(ko == 0),
                                stop=(ko == KT - 1),
                            )
                        nc.vector.tensor_copy(kvl_sb[:, lo, :], ps[:])
                    fill()

                    # K^T per head (+rope)
                    for hh in range(GH):
                        ps = ps1.tile([128, SB], F32, tag="ps1")
                        for lo in range(LT):
                            nc.tensor.matmul(
                                ps[:],
                                wuk_sb[:, lo, hh * 128:(hh + 1) * 128],
                                kvl_sb[:, lo, :],
                                start=(lo == 0),
                                stop=(lo == LT - 1),
                            )
                        rope(kT_c[sb][:, hh, :], ps, tmp1, s0, SB)
                    fill()

                    # V in [s, d] layout: s-chunks of 128
                    for sc in range(4):
                        ps = ps1.tile([128, GD], F32, tag="ps1")
                        for lo in range(LT):
                            nc.tensor.matmul(
                                ps[:],
                                kvl_sb[:, lo, sc * 128:(sc + 1) * 128],
                                wuv_sb[:, lo, :],
                                start=(lo == 0),
                                stop=(lo == LT - 1),
                            )
                        nc.scalar.copy(
                            v_c[sb][:, sc, :, :HD],
                            ps.rearrange("p (h d) -> p h d", h=GH),
                        )
                    fill()

                    q_section()

                def attn(qi, heads=range(GH)):
                    njb = 4 * qi + 4  # causal: k-blocks 0..4*qi+3

                    def qk_exp(j, hh):
                        """logits + exp(+causal mask) for k-block j."""
                        t = j - 4 * qi
                        # causal: q-cols < 128*t fully masked
                        qoff = 0 if t < 1 else 128 * t
                        nw = QC - qoff
                        lg = pslg.tile([128, QC], F32, tag="lg")
                        nc.tensor.matmul(
                            lg[:, :nw],
                            kT_c[j // 4][:, hh,
                                         (j % 4) * 128:(j % 4 + 1) * 128],
                            qT_c[qi][:, hh, qoff:],
                            start=True,
                            stop=True,
                        )
                        e_sb = attp.tile([128, QC], F16, tag="e")
                        nc.scalar.activation(
                            e_sb[:, :nw],
                            lg[:, :nw],
                            mybir.ActivationFunctionType.Exp,
                            scale=SCALE,
                        )
                        if t >= 0:
                            m0 = 384 - 128 * t + qoff
                            nc.vector.tensor_mul(
                                e_sb[:, :nw], e_sb[:, :nw],
                                masks_sb[:, m0:m0 + nw],
                            )
                        return e_sb

                    for hh in heads:
                        # [128,129] accumulator per 128-wide q-sub, packed
                        # in pairs so each stays inside one PSUM bank:
                        # cols 0:128 = O (q rows, d cols), col 128 = R
                        oqA = psot.tile([128, 2 * (HD + 1)], F32, tag="oqA",
                                        bufs=1, name="oqA")
                        oqB = psot.tile([128, 2 * (HD + 1)], F32, tag="oqB",
                                        bufs=1, name="oqB")
                        o_qs = [oqA[:, 0:HD + 1], oqA[:, HD + 1:],
                                oqB[:, 0:HD + 1], oqB[:, HD + 1:]]
                        # software pipeline: QK/exp two k-blocks ahead of AV
                        e_q = [qk_exp(jj, hh) for jj in range(min(2, njb))]
                        for j in range(njb):
                            t = j - 4 * qi
                            qoff = 0 if t < 1 else 128 * t
                            e_sb = e_q.pop(0)
                            if j + 2 < njb:
                                e_q.append(qk_exp(j + 2, hh))
                            for s in range(max(t, 0), 4):
                                # start=True clears the whole PSUM bank, so
                                # only the bank's first matmul (s even at
                                # j=0) gets it; the odd-s group's first
                                # write lands on has_written=0 elements and
                                # overwrites rather than accumulates.
                                nc.tensor.matmul(
                                    o_qs[s][:, :],
                                    e_sb[:, s * 128 - qoff:
                                         (s + 1) * 128 - qoff],
                                    v_c[j // 4][:, j % 4, hh, :],
                                    start=(j == 0 and s % 2 == 0),
                                    stop=(j == 4 * qi + s),
                                )
                        # normalize rows: O[q, :] / R[q] (per-partition
                        # scalar); runs on ACT so the next head's AV can
                        # start into the other PSUM buffer immediately
                        for s in range(4):
                            rec = attp.tile([128, 1], F32, tag="rec")
                            nc.vector.reciprocal(rec[:], o_qs[s][:, HD:])
                            nc.scalar.mul(
                                o_c[qi][:, s, hh * HD:(hh + 1) * HD],
                                o_qs[s][:, :HD],
                                rec[:],
                            )

                # ---- stage 1: s-blocks with attention chunks woven in ----
                for sb in range(NSB):
                    if sb + 1 < NSB:
                        # xq8 first: the next block opens with its q section
                        nxq = xtp.tile([128, KT, SB], F8, tag="xq",
                                       name=f"xq{sb + 1}")
                        xq_sbs[sb + 1] = nxq
                        nc.sync.dma_start(
                            nxq[:], xq8_t[:, :, (sb + 1) * SB:(sb + 2) * SB])
                        nxt = xtp.tile([128, KT, SB], F16, tag="xt",
                                       name=f"xt{sb + 1}")
                        xt_sbs[sb + 1] = nxt
                        for kg in range(4):
                            nc.sync.dma_start(
                                nxt[:, 4 * kg:4 * kg + 4, :],
                                xT_t[:, 4 * kg:4 * kg + 4,
                                     (sb + 1) * SB:(sb + 2) * SB],
                            )
                    if sb == 0:
                        proj_block(sb)
                    else:
                        proj_block(sb, [
                            (lambda h: lambda: attn(sb - 1, heads=[h]))(h)
                            for h in range(GH)
                        ])

                if stage <= 1:
                    nc.sync.dma_start(pout[0:128, 0:516], v_c[0][:, 0, :, :])
                    nc.sync.dma_start(pout[128:256, 0:512],
                                      kT_c[0][:, 0, 0:512])
                    ps1.release()
                    cs.release()
                    xtp.release()
                    return

                # ---- stage 2: attn(3) + all output projections fused ----
                ps1.release()
                cs.release()
                xtp.release()
                with (
                    tc.tile_pool(name="w3", bufs=1) as w3,
                    tc.tile_pool(name="otr", bufs=2) as otrp,
                    tc.tile_pool(name="osb", bufs=4) as osbp,
                    tc.tile_pool(name="ps3", bufs=1, space="PSUM",
                                 side="right") as ps3,
                ):
                    wp_sb = w3.tile([128, LT, DOUT], F16)
                    for dt_ in range(LT):
                        nc.sync.dma_start(wp_sb[:, dt_, :], wp_t[:, dt_, :])

                    def ph3(qi, sc):
                        q0 = qi * QC
                        tr = ps3.tile([128, 512], F16, tag="tr", bufs=2)
                        for dt_ in range(LT):
                            nc.tensor.transpose(
                                tr[:, dt_ * HD:(dt_ + 1) * HD],
                                o_c[qi][:, sc, dt_ * HD:(dt_ + 1) * HD],
                                eye_sb[:],
                            )
                        # after attention drains, ACT is idle: split the
                        # tail chunk's PSUM evacuations across ACT and DVE
                        def evac(o, i, k):
                            if qi == 3 and k % 2 == 0:
                                nc.scalar.copy(o, i)
                            else:
                                nc.vector.tensor_copy(o, i)
                        oT_sb = otrp.tile([128, 512], F16, tag="ot")
                        evac(oT_sb[:], tr[:], 0)
                        out_sb = osbp.tile([128, DOUT], F16, tag="out")
                        for ec in range(DOUT // 512):
                            po = ps3.tile([128, 512], F32, tag="po", bufs=2)
                            for dt_ in range(LT):
                                nc.tensor.matmul(
                                    po[:],
                                    oT_sb[:, dt_ * HD:(dt_ + 1) * HD],
                                    wp_sb[:, dt_, ec * 512:(ec + 1) * 512],
                                    start=(dt_ == 0),
                                    stop=(dt_ == LT - 1),
                                )
                            evac(out_sb[:, ec * 512:(ec + 1) * 512], po[:],
                                 ec + 1)
                        nc.sync.dma_start(
                            pout[q0 + sc * 128:q0 + (sc + 1) * 128, :],
                            out_sb[:],
                        )

                    # attn(3) heads interleaved with the already-finished
                    # chunks' output projections (fills ACT-bound PE gaps)
                    units = [(qi, sc) for qi in range(3) for sc in range(4)]
                    for x in range(4):
                        attn(3, heads=[x])
                        for qi, sc in units[3 * x:3 * x + 3]:
                            ph3(qi, sc)
                    for sc in range(4):
                        ph3(3, sc)
            # (kvres/persist close)


_CACHE: dict = {}


def _get_nc():
    if "nc" not in _CACHE:
        _CACHE["nc"] = build_nc()
    return _CACHE["nc"]


def _host_inputs(x, position_embeddings, Wq, Wl, Wu, Wp):
    x = np.asarray(x, dtype=np.float32)
    pe = np.asarray(position_embeddings, dtype=np.float32)[:S]
    Wq = np.asarray(Wq, dtype=np.float32)
    Wl = np.asarray(Wl, dtype=np.float32)
    Wu = np.asarray(Wu, dtype=np.float32)
    Wp = np.asarray(Wp, dtype=np.float32)

    cos = np.ascontiguousarray(np.cos(pe).T)          # [HD, S]
    sinF = np.ascontiguousarray(np.sin(pe).T)         # [HD, S]
    sinF[: HD // 2] *= -1.0                           # fold rotate-half sign

    k = np.arange(128)[:, None]
    c = np.arange(QC + 384)[None, :]
    masks = np.ascontiguousarray((c - 384 >= k).astype(np.float16))

    import ml_dtypes

    f8 = ml_dtypes.float8_e4m3
    xTs = [np.ascontiguousarray(x[b].T.astype(np.float16)) for b in range(B)]
    xq8s = [np.ascontiguousarray(x[b].T.astype(f8)) for b in range(B)]
    wl16 = np.ascontiguousarray(Wl.astype(np.float16))

    in_maps = []
    for cid in range(NCORES):
        b, g = divmod(cid, GROUPS)
        in_maps.append({
            "xT": xTs[b],
            "xq8": xq8s[b],
            "wq": np.ascontiguousarray(
                Wq[:, g * GD:(g + 1) * GD].astype(f8)),
            "wl": wl16,
            "wuk": np.ascontiguousarray(
                Wu[:, g * GD:(g + 1) * GD].astype(np.float16)),
            "wuv": np.ascontiguousarray(
                Wu[:, DOUT + g * GD:DOUT + (g + 1) * GD].astype(np.float16)),
            "wp": np.ascontiguousarray(
                Wp[g * GD:(g + 1) * GD, :].astype(np.float16)),
            "cosT": cos,
            "sinT": sinF,
            "masks": masks,
            "eye": np.eye(128, dtype=np.float16),
        })
    return in_maps


def run(x, position_embeddings, Wq, Wl, Wu, Wp, trace=False, trace_cores=None):
    """Run on 8 cores; returns (output, BassKernelResults)."""
    nc = _get_nc()
    in_maps = _host_inputs(x, position_embeddings, Wq, Wl, Wu, Wp)
    if trace and trace_cores is None:
        trace_cores = list(range(NCORES))
    res = bass_utils.run_bass_kernel_spmd(
        nc, in_maps, core_ids=list(range(NCORES)), trace=trace,
        trace_cores=trace_cores,
    )
    parts = [r["pout"].astype(np.float32) for r in res.results]
    out = np.empty((B, S, DOUT), dtype=np.float32)
    for b in range(B):
        out[b] = np.sum(
            np.stack(parts[b * GROUPS:(b + 1) * GROUPS]),
            axis=0, dtype=np.float64,
        ).astype(np.float32)
    return out, res


def kernel(x, position_embeddings, Wq, Wl, Wu, Wp):
    out, _ = run(x, position_embeddings, Wq, Wl, Wu, Wp, trace=False)
    return out
